# revision 24
# baseline (speedup 1.0000x reference)
"""Bahdanau additive attention kernel for Trainium2 (8 NeuronCores).

Reference computation (B=32, S=4096, D=512):
    pre   = enc @ We.T + (hidden @ Wh.T + b1)[:, None, :]   # [B, S, D]
    h     = tanh(pre)
    e     = h @ w2                                          # [B, S]
    alpha = softmax(e, axis=1)
    ctx   = einsum('bs,bsd->bd', alpha, enc)                # [B, D]

Strategy: data-parallel over batch (4 batches per core). Inputs are
re-laid-out on host so the device only ever does efficient, contiguous
DMA:
  - enc is passed transposed per-batch as [st, p, di, s] bf16 so the
    contraction dim d sits on SBUF partitions for the PE matmuls AND
    the per-(b,st) tile DMA is a pure identity copy (128 descriptors
    of 8KB).
  - The big matmul runs in bf16 (1 cycle/row on PE vs 4 for fp32);
    accumulation is fp32 in PSUM. Verified rel-l2 error ~2.6e-3.
    Loop order ki->di->half reuses each stationary weight block for
    both 512-wide halves (one LDWEIGHTS per 2 matmuls).
  - The per-batch bias c = hidden @ Wh.T + b1 (0.004% of the FLOPs) is
    computed on host during input prep and applied as the per-partition
    ACT bias of the fused tanh.
  - PE warm-up matmuls run on memset tiles so they start right after
    the preamble; enc tile DMAs go on the gpsimd queue so the first
    tile streams concurrently with the weight DMA on the sync queue.
  - e rows are accumulated in PSUM via M=1 matmuls with w2 stationary;
    exp runs streamed/UNNORMALIZED (|e| < ~5 so no max pass), with the
    row-sum fused via accum_out. The 1/sum normalization is applied
    once per batch to the final [128, NDC] context.
  - each exp'd 512-wide row round-trips through DRAM to partition-
    broadcast, then p * enc is accumulated on the DVE
    (scalar_tensor_tensor with fused free-dim reduce). For the final
    s-tile of the final batch the broadcast instead runs as a PE
    ones-matmul into PSUM (the PE is idle by then) so the exposed
    end-of-kernel chain skips the ~6us DMA round trip.
  - per-batch context is staged in SBUF, compressed to [16, 128] via
    DVE 32x32 block transposes, and written with one 16-descriptor
    output DMA (a [128, x] SBUF source costs 128 tiny descriptors on
    one ring, ~6us of drain).
"""

import sys

if "/opt/trn_rl_repo" not in sys.path:
    sys.path.insert(0, "/opt/trn_rl_repo")

from contextlib import ExitStack

import ml_dtypes
import numpy as np

import concourse.bass as bass
import concourse.bacc as bacc
import concourse.tile as tile
from concourse import mybir
from concourse.bass_utils import run_bass_kernel_spmd

B, S, D = 32, 4096, 512
NCORES = 8
BPC = B // NCORES          # batches per core
P = 128                    # partitions
NDC = D // P               # d (contraction) chunks
NKC = D // P               # k (output channel) chunks
ST = 1024                  # s-tile size (PE/ACT granularity)
NST = S // ST              # s tiles per batch
EST = 512                  # e-row granularity (one PSUM bank / DVE tile)
NER = S // EST             # exp rows per batch
NWARM = 26                 # PE warm-up matmuls (N=512 each)

F32 = mybir.dt.float32
BF16 = mybir.dt.bfloat16
AF = mybir.ActivationFunctionType
ALU = mybir.AluOpType


def build_bass():
    nc = bacc.Bacc()

    encT = nc.declare_dram_parameter("encT", [BPC, NST, P, NDC, ST], BF16, isOutput=False)
    weT = nc.declare_dram_parameter("weT", [P, NDC, D], BF16, isOutput=False)
    cb = nc.declare_dram_parameter("cb", [P, NKC, BPC], F32, isOutput=False)
    w2r = nc.declare_dram_parameter("w2r", [P, NKC], BF16, isOutput=False)
    ctx_out = nc.declare_dram_parameter("ctx", [BPC * NDC, P], F32, isOutput=True)

    with TileKernel(nc) as tk:
        tk.build(encT, weT, cb, w2r, ctx_out)
    nc.finalize()
    return nc


class TileKernel:
    def __init__(self, nc):
        self.nc = nc
        self.stack = ExitStack()
        self.tc = None

    def __enter__(self):
        self.tc = self.stack.enter_context(tile.TileContext(self.nc))
        return self

    def __exit__(self, *exc):
        return self.stack.__exit__(*exc)

    def build(self, encT, weT, cb, w2r, ctx_out):
        nc, tc, ctx = self.nc, self.tc, self.stack

        singles = ctx.enter_context(tc.tile_pool(name="singles", bufs=1))
        encp = ctx.enter_context(tc.tile_pool(name="encp", bufs=2 * NST))
        htp = ctx.enter_context(tc.tile_pool(name="htp", bufs=4))
        smp = ctx.enter_context(tc.tile_pool(name="smp", bufs=3))
        dramp = ctx.enter_context(tc.tile_pool(name="dramp", bufs=2, space="DRAM"))
        # 5x 1-bank pre tiles + 2x e-row banks + 1 broadcast bank = 8 banks
        psump = ctx.enter_context(tc.tile_pool(name="psump", bufs=5, space="PSUM"))

        # ---- PE warm-up burst on memset tiles (no DMA dependency) ----
        # ~3.4us of sustained matmul activity flips the HAM clock gate to
        # 8/8 while the weight/enc DMAs are still in flight.
        wl = singles.tile([P, 1], BF16)
        nc.vector.memset(wl, 1.0)
        wr = singles.tile([P, EST], BF16)
        nc.vector.memset(wr, 0.125)
        ones1 = singles.tile([1, P], BF16)
        nc.vector.memset(ones1, 1.0)
        for i in range(NWARM):
            wpre = psump.tile([1, EST], F32, tag="ec", bufs=2)
            nc.tensor.matmul(out=wpre, lhsT=wl, rhs=wr, start=True, stop=True)
        wjunk = singles.tile([1, 1], F32)
        nc.vector.tensor_copy(out=wjunk, in_=wpre[:, 0:1])

        # ---- load constants (identity-layout DMAs, sync queue) ----
        c_sb = singles.tile([P, NKC, BPC], F32)
        nc.sync.dma_start(out=c_sb, in_=cb[:])
        w2_sb = singles.tile([P, NKC], BF16)
        nc.sync.dma_start(out=w2_sb, in_=w2r[:])
        w_sb = singles.tile([P, NDC, D], BF16)
        nc.sync.dma_start(out=w_sb, in_=weT[:])

        stage = singles.tile([P, 32], F32)
        nc.vector.memset(stage, 0.0)

        # ---- main per-batch pipeline ----
        for b in range(BPC):
            pd = dramp.tile([NER, EST], BF16, tag="pd")
            lparts = smp.tile([1, NER], F32, tag="lparts")
            cacc = smp.tile([P, NDC, NER], F32, tag="cacc")
            rinvb = smp.tile([P, 1], F32, tag="rinvb")
            for st in range(NST):
                et = encp.tile([P, NDC, ST], BF16, tag="et")
                nc.sync.dma_start(out=et, in_=encT[:][b, st])

                ht = htp.tile([P, NKC, ST], BF16, tag="ht")
                for ki in range(NKC):
                    pre_h = [psump.tile([P, EST], F32, tag="pre", name=f"pre{h}")
                             for h in range(ST // EST)]
                    for di in range(NDC):
                        for half in range(ST // EST):
                            sl = slice(half * EST, (half + 1) * EST)
                            nc.tensor.matmul(
                                out=pre_h[half],
                                lhsT=w_sb[:, di, ki * P:(ki + 1) * P],
                                rhs=et[:, di, sl],
                                start=(di == 0),
                                stop=(di == NDC - 1),
                            )
                    # h^T = tanh(pre^T + c), one [128, EST] ACT op per half
                    for half in range(ST // EST):
                        sl = slice(half * EST, (half + 1) * EST)
                        nc.scalar.activation(
                            out=ht[:, ki, sl],
                            in_=pre_h[half],
                            func=AF.Tanh,
                            bias=c_sb[:, ki, b:b + 1],
                            scale=1.0,
                        )
                last_tile = st == NST - 1 and b == BPC - 1
                for half in range(ST // EST):
                    sl = slice(half * EST, (half + 1) * EST)
                    r = st * (ST // EST) + half
                    e_ps = psump.tile([1, EST], F32, tag="ec", bufs=2)
                    for ki in range(NKC):
                        nc.tensor.matmul(
                            out=e_ps,
                            lhsT=w2_sb[:, ki:ki + 1],
                            rhs=ht[:, ki, sl],
                            start=(ki == 0),
                            stop=(ki == NKC - 1),
                        )
                    # p = exp(e) with the row-sum fused
                    p_row = smp.tile([1, EST], BF16, tag="prow")
                    nc.scalar.activation(
                        out=p_row, in_=e_ps, func=AF.Exp, bias=0.0, scale=1.0,
                        accum_out=lparts[:, r:r + 1],
                    )
                    if r == NER - 1:
                        # the 1/sum chain only needs the exps; emit it ahead
                        # of the last STTs so it overlaps them.
                        lsum = smp.tile([1, 1], F32, tag="lsum")
                        nc.vector.reduce_sum(
                            out=lsum, in_=lparts, axis=mybir.AxisListType.X)
                        rinv1 = smp.tile([1, 1], F32, tag="rinv1")
                        nc.vector.reciprocal(out=rinv1, in_=lsum)
                        nc.gpsimd.partition_broadcast(out_ap=rinvb, in_ap=rinv1)
                    if last_tile:
                        # PE is idle by now: broadcast p across partitions
                        # with a rank-1 ones-matmul instead of the DMA
                        # round trip (saves ~6us of exposed tail latency).
                        ab_ps = psump.tile([P, EST], F32, tag="abps", bufs=1)
                        nc.tensor.matmul(
                            out=ab_ps, lhsT=ones1, rhs=p_row,
                            start=True, stop=True,
                        )
                        ab = ab_ps
                    else:
                        nc.sync.dma_start(out=pd[r:r + 1, :], in_=p_row)
                        # broadcast this 512-wide p row across partitions
                        ab = htp.tile([P, EST], BF16, tag="ab", bufs=6)
                        row = pd[r:r + 1, :]
                        nc.sync.dma_start(
                            out=ab,
                            in_=bass.AP(
                                tensor=row.tensor,
                                offset=row.offset,
                                ap=[[0, P], [1, EST]],
                            ),
                        )
                    for di in range(NDC):
                        junk = htp.tile([P, EST], BF16, tag="junk", bufs=3)
                        nc.vector.scalar_tensor_tensor(
                            out=junk,
                            in0=et[:, di, sl],
                            scalar=1.0,
                            in1=ab,
                            op0=ALU.mult,
                            op1=ALU.mult,
                            accum_out=cacc[:, di, r:r + 1],
                        )

            # ---- finalize: stage[:, b*4:(b+1)*4] = (sum_s p*enc)/sum_s p ----
            # (rinvb was already produced right after the last exp above)
            ctx_acc = smp.tile([P, NDC], F32, tag="ctx")
            nc.vector.reduce_sum(out=ctx_acc, in_=cacc, axis=mybir.AxisListType.X)
            nc.vector.tensor_scalar_mul(
                out=stage[:, b * NDC:(b + 1) * NDC], in0=ctx_acc, scalar1=rinvb
            )

        # compress [128, 16] -> [16, 128] via 32x32 block transposes, then one
        # 16-descriptor output DMA.
        comp = singles.tile([32, P], F32)
        for j in range(4):
            nc.vector.transpose(
                out=comp[:, 32 * j:32 * (j + 1)],
                in_=stage[32 * j:32 * (j + 1), :],
            )
        nc.sync.dma_start(out=ctx_out[:], in_=comp[0:BPC * NDC, :])


_NC_CACHE = None


def _get_nc():
    global _NC_CACHE
    if _NC_CACHE is None:
        _NC_CACHE = build_bass()
    return _NC_CACHE


def _prep_core_inputs(hidden_state, encoder_outputs, W1, b1, w2, core):
    bf16 = ml_dtypes.bfloat16
    b0 = core * BPC
    enc = encoder_outputs[b0:b0 + BPC]                      # [BPC, S, D] f32
    # [b, d, s] -> [b, di, p, st, s] -> [b, st, p, di, s]
    e = enc.transpose(0, 2, 1).reshape(BPC, NDC, P, NST, ST)
    e = np.ascontiguousarray(e.transpose(0, 3, 2, 1, 4)).astype(bf16)
    # c = hidden @ Wh.T + b1  (tiny: 0.004% of model FLOPs, done during prep)
    c = hidden_state[b0:b0 + BPC] @ W1[:, D:].T + b1        # [BPC, D]
    c = np.ascontiguousarray(c.reshape(BPC, NKC, P).transpose(2, 1, 0))
    return {
        "encT": e,
        "weT": np.ascontiguousarray(
            W1[:, :D].T.reshape(NDC, P, D).transpose(1, 0, 2)).astype(bf16),
        "cb": c,
        "w2r": np.ascontiguousarray(w2.reshape(NKC, P).T).astype(bf16),
    }


def kernel(hidden_state, encoder_outputs, W1, b1, w2, _trace=False, _trace_kwargs=None):
    hidden_state = np.asarray(hidden_state, dtype=np.float32)
    encoder_outputs = np.asarray(encoder_outputs, dtype=np.float32)
    W1 = np.asarray(W1, dtype=np.float32)
    b1 = np.asarray(b1, dtype=np.float32)
    w2 = np.asarray(w2, dtype=np.float32)

    nc = _get_nc()
    in_maps = [
        _prep_core_inputs(hidden_state, encoder_outputs, W1, b1, w2, c)
        for c in range(NCORES)
    ]
    res = run_bass_kernel_spmd(
        nc, in_maps, list(range(NCORES)), trace=_trace,
        **(_trace_kwargs or {}),
    )
    out = np.empty((B, D), dtype=np.float32)
    for c in range(NCORES):
        r = res.results[c]["ctx"]                          # [b*di, p]
        out[c * BPC:(c + 1) * BPC] = r.reshape(BPC, D)
    if _trace:
        return out, res
    return out
